# revision 9
# baseline (speedup 1.0000x reference)
"""Dynamic per-pixel depthwise 3x3 conv (DYDConv2d) on 8 Trainium2 cores.

Full-tensor contract:
    input : (8, 64, 128, 128) f32
    weight: (8, 64, 3, 3, 128, 128) f32   -- one 3x3 filter per (b, c, oh, ow)
    out   : (8, 64, 128, 128) f32
    out[b,c,oh,ow] = sum_{i,j} xpad[b,c,oh+i,ow+j] * weight[b,c,i,j,oh,ow]
    (stride 1, pad 1)

Sharding: data-parallel over batch B=8 -> one sample per NeuronCore.

v2 design (fp16 + PE accumulation), from the v1 post-mortem: v1 (f32,
all-DVE) sat at ~104 us, simultaneously at the DVE 1x-mode elementwise
limit (34 ops x 4096 FD) and the ~435 GB/s DMA fabric limit (46 MB/core).
Both walls halve in fp16 (harness gate is 2e-2; fp16 end-to-end measures
5e-4): weights stream as 18.9 MB, and DVE tensor_tensor runs in 2x_1P
packed mode.  The 8 accumulate-adds move off DVE entirely: the idle
TensorEngine sums the 9 tap-products into PSUM via identity-matrix
matmuls (PSUM accumulate), so DVE only does the 9 multiplies.  The
kh*kw j=1 taps break 2x-mode's 4B-alignment rule, so the idle ScalarE
makes a column-shifted copy of the x slab once per pass; ScalarE also
evacuates PSUM (f32) to fp16 for a half-size output stream.

Per-core layout: 128 SBUF partitions = (channel c in 0..63) x (H-half hf in
{0,1}), partition p = c*2 + hf.  Each partition holds a (66 x 130) zero-
padded fp16 slab of its half-image.  Output rows are processed in 4 chunks
of 16 rows (FD=2048 = 4 PSUM banks, double-buffered = all 8 banks); each
chunk's 9 weight tiles arrive as ONE contiguous 4.7 MB DMA from a
host-pretransposed [chunk][part][tap][row][col] fp16 stream.

Measured steady state ~48.5 us/core/pass = 21.07 MB inbound at 434 GB/s,
i.e. 100% of the 16-port x 32B x 850 MHz = 435 GB/s SBUF-AXI inbound
ceiling (outbound overlaps; engines all below the wall: DVE 9 mults/chunk
in 2x mode, PE 144 identity-matmuls, ACT shift-copy + 4 PSUM evacs).
Going below needs fewer weight bytes: fp8/hybrid-fp8 measured 2.2-2.7e-2
max-rel error (gate 2e-2) and int8-blockfloat needs an upconvert that
costs more engine time than the DMA it saves (PE rejects int8 moving
data; 8-bit operands drop DVE to 1x mode) -- both dead ends, so 16-bit
weights are the floor.
"""

import numpy as np

import concourse.bacc as bacc
import concourse.mybir as mybir
from concourse.bass_utils import run_bass_kernel_spmd
from concourse.tile import TileContext

B, C, H, W = 8, 64, 128, 128
KH, KW = 3, 3
NTAP = KH * KW
HALF = H // 2  # rows per half-image (one partition group)
SLAB_R, SLAB_C = HALF + 2, W + 2  # 66 x 130 padded slab per partition

RT = 16                      # output rows per chunk -> FD=2048 = 4 PSUM banks
N_CHUNKS = HALF // RT        # 4
FD = RT * W                  # 2048
BANK = 512                   # f32 elements per PSUM bank
NBANK = FD // BANK           # 4

_F16 = mybir.dt.float16
_F32 = mybir.dt.float32


def _emit(nc, tc, xs, w, ident, o, rep=1, mode="pe"):
    """Per-core program.

    xs   : [128, SLAB_R*SLAB_C] f16   zero-padded slab (host-built)
    w    : [N_CHUNKS, 128, NTAP*FD] f16  host-pretransposed weight stream
    ident: [128, 128] f16             identity for PE accumulate
    o    : [N_CHUNKS, 128, FD] f16    chunk-major output
    rep > 1 repeats the complete pass back-to-back (steady-state timing).
    """
    with tc.tile_pool(name="const", bufs=1) as cpool:
        idt = cpool.tile([128, 128], _F16, name="idt")
        nc.sync.dma_start(out=idt[:], in_=ident[:])

        with tc.tile_pool(name="slab", bufs=2) as spool, \
             tc.tile_pool(name="work", bufs=2) as pool, \
             tc.tile_pool(name="psum", bufs=2, space="PSUM") as ppool:
            for _r in range(rep):
                xbuf = spool.tile([128, SLAB_R, SLAB_C], _F16, name="xbuf")
                nc.sync.dma_start(
                    out=xbuf[:].rearrange("p r cc -> p (r cc)"), in_=xs[:]
                )

                # column-shifted copy: xsh[p,r,c] = xbuf[p,r,c+1]; gives the
                # j=1 taps a 4B-aligned source so DVE stays in 2x mode.
                xsh = spool.tile([128, SLAB_R, W], _F16, name="xsh")
                nc.scalar.copy(out=xsh[:], in_=xbuf[:, :, 1 : W + 1])

                def xtap(k, t):
                    i, j = divmod(t, KW)
                    r0 = k * RT + i
                    if j == 1:
                        return xsh[:, r0 : r0 + RT, :]
                    return xbuf[:, r0 : r0 + RT, j : j + W]

                for k in range(N_CHUNKS):
                    wt = pool.tile([128, NTAP, RT, W], _F16, name="wt")
                    nc.sync.dma_start(
                        out=wt[:].rearrange("p t r ww -> p (t r ww)"),
                        in_=w[k],
                    )
                    if mode == "pe":
                        ps = ppool.tile([128, FD], _F32, name="ps")
                        for t in range(NTAP):
                            prod = pool.tile(
                                [128, RT, W], _F16, name="prod", bufs=3
                            )
                            nc.vector.tensor_tensor(
                                prod[:], xtap(k, t), wt[:, t],
                                mybir.AluOpType.mult,
                            )
                            pv = prod[:].rearrange("p r ww -> p (r ww)")
                            for bk in range(NBANK):
                                nc.tensor.matmul(
                                    ps[:, bk * BANK : (bk + 1) * BANK],
                                    idt[:],
                                    pv[:, bk * BANK : (bk + 1) * BANK],
                                    start=(t == 0),
                                    stop=(t == NTAP - 1),
                                )
                        osb = pool.tile([128, FD], _F16, name="osb")
                        nc.scalar.copy(out=osb[:], in_=ps[:])
                        nc.scalar.dma_start(out=o[k], in_=osb[:])
                    else:  # mode == "dve": fp16 mult+add chains, no PE
                        acc = pool.tile([128, RT, W], _F16, name="acc")
                        tmp = pool.tile([128, RT, W], _F16, name="tmp", bufs=1)
                        nc.vector.tensor_tensor(
                            acc[:], xtap(k, 0), wt[:, 0], mybir.AluOpType.mult
                        )
                        for t in range(1, NTAP):
                            nc.vector.tensor_tensor(
                                tmp[:], xtap(k, t), wt[:, t],
                                mybir.AluOpType.mult,
                            )
                            nc.vector.tensor_tensor(
                                acc[:], acc[:], tmp[:], mybir.AluOpType.add
                            )
                        nc.scalar.dma_start(
                            out=o[k],
                            in_=acc[:].rearrange("p r ww -> p (r ww)"),
                        )


def build_program(rep=1, mode="pe"):
    nc = bacc.Bacc(
        "TRN2",
        target_bir_lowering=False,
        debug=False,
        enable_asserts=False,
        num_devices=8,
    )
    xs = nc.dram_tensor(
        "xs", [128, SLAB_R * SLAB_C], _F16, kind="ExternalInput"
    ).ap()
    w = nc.dram_tensor(
        "w", [N_CHUNKS, 128, NTAP * FD], _F16, kind="ExternalInput"
    ).ap()
    ident = nc.dram_tensor("ident", [128, 128], _F16, kind="ExternalInput").ap()
    o = nc.dram_tensor("o", [N_CHUNKS, 128, FD], _F16, kind="ExternalOutput").ap()
    with TileContext(nc) as tc:
        _emit(nc, tc, xs, w, ident, o, rep=rep, mode=mode)
    nc.compile()
    return nc


def make_slab(x_one):
    """Host-side zero-padded fp16 slab: [64,128,128] -> [128, 66*130].

    Partition p = c*2 + hf holds rows hf*64-1 .. hf*64+64 of channel c
    (zero-padded at the image border) in a 66x130 col-padded layout.
    """
    slab = np.zeros((C, 2, SLAB_R, SLAB_C), dtype=np.float16)
    slab[:, 0, 1 : HALF + 2, 1 : W + 1] = x_one[:, 0 : HALF + 1, :]
    slab[:, 1, 0 : HALF + 1, 1 : W + 1] = x_one[:, HALF - 1 : H, :]
    return slab.reshape(128, SLAB_R * SLAB_C)


def make_wstream(w_one):
    """[64,3,3,128,128] f32 -> [N_CHUNKS, 128, NTAP*FD] f16 chunk-major.

    w_stream[k, c*2+hf, ((t*RT)+r)*W + ow] = w_one[c, t//3, t%3,
                                                   hf*HALF + k*RT + r, ow]
    """
    v = w_one.reshape(C, NTAP, 2, N_CHUNKS, RT, W)
    v = np.transpose(v, (3, 0, 2, 1, 4, 5))  # k, c, hf, t, r, ow
    return np.ascontiguousarray(
        v.reshape(N_CHUNKS, 128, NTAP * FD), dtype=np.float16
    )


def unmake_out(o_core):
    """[N_CHUNKS, 128, FD] f16 -> [64, 128, 128] f32."""
    v = o_core.reshape(N_CHUNKS, C, 2, RT, W).astype(np.float32)
    v = np.transpose(v, (1, 2, 0, 3, 4))  # c, hf, k, r, ow
    return v.reshape(C, H, W)


_IDENT = np.eye(128, dtype=np.float16)
_CACHE = {}


def kernel(input, weight, _trace=False):
    input = np.asarray(input, dtype=np.float32)
    weight = np.asarray(weight, dtype=np.float32)
    assert input.shape == (B, C, H, W), input.shape
    assert weight.shape == (B, C, KH, KW, H, W), weight.shape

    if "nc" not in _CACHE:
        _CACHE["nc"] = build_program()
    nc = _CACHE["nc"]

    in_maps = [
        {
            "xs": make_slab(input[b]),
            "w": make_wstream(weight[b]),
            "ident": _IDENT,
        }
        for b in range(B)
    ]
    res = run_bass_kernel_spmd(nc, in_maps, core_ids=list(range(B)), trace=_trace)
    _CACHE["last_result"] = res
    out = np.stack([unmake_out(res.results[b]["o"]) for b in range(B)], axis=0)
    return out.astype(np.float32, copy=False)
